# revision 5
# baseline (speedup 1.0000x reference)
"""Trainium2 Bass kernel for the nn_HVAE loss function, SPMD over 8 NeuronCores.

Sharding: edge_logits (8192x8192) and the z_* (8192x64) tensors are row-sharded
over nodes across the 8 cores.  Per core:

  BCE dense term   sum(softplus(x)) = sum(ln(1 + e^x)): ScalarE computes
                   e = exp(x); VectorE pairs columns (1+e_lo)(1+e_hi) - 1 so
                   ScalarE's Ln(u + 1) pass runs on half the elements; the
                   per-partition sums come from the activation accumulator.
                   Exp and Ln share one activation table set
                   (natural_log_exp_and_others) to avoid table reloads.
  BCE edge term    sum(x * adj): the host dedups + symmetrizes edge_index into
                   per-row column lists; GPSIMD ap_gather pulls the edge
                   entries of x into a compact [128, 16*KH] tile (16-row
                   groups, fixed KH-wide per-row segments), a tensor_scalar
                   range-compare builds the validity mask, and VectorE does the
                   masked dot-product + reduction.
  KL terms         over the row-sharded z tensors on ScalarE/VectorE.

Each core writes per-partition partial sums; the host does the final (tiny)
all-reduce of the scalar partials in float64.
"""

import sys

import numpy as np

sys.path.insert(0, "/opt/trn_rl_repo")

import concourse.bass as bass  # noqa: E402
import concourse.tile as tile  # noqa: E402
from concourse import bacc, hw_specs, mybir  # noqa: E402
from concourse.bass_utils import run_bass_kernel_spmd  # noqa: E402

N = 8192          # num nodes
D = 64            # latent dim
M = 8             # cores
R = N // M        # 1024 rows per core
RT = R // 128     # 8 row tiles of 128 partitions per core
H = N // 2        # 4096: pairing half-width
ZW = 6 * (R // 128) * D   # 3072: six (1024,64) pairs packed as [128, 512] each
ZC = 3            # z chunks
ZWc = ZW // ZC    # 1024
FP16_PAD = 32768.0  # iota2 fill for foreign segments (never < count)
LOG2PI = float(np.log(2.0 * np.pi))
LV = float(np.log(0.25))

f32 = mybir.dt.float32
bf16 = mybir.dt.bfloat16
fp16 = mybir.dt.float16
i16 = mybir.dt.int16

_PROG_CACHE: dict = {}

Z_PAIRS = [
    ("z_mu_e", "z_logvar_e", "mu_Alpha"),
    ("z_mu_n", "z_logvar_n", "mu_Beta"),
    ("z_mu_e1", "z_logvar_e1", "mu_Alpha1"),
    ("z_mu_n1", "z_logvar_n1", "mu_Beta1"),
    ("z_mu_e2", "z_logvar_e2", "mu_Alpha2"),
    ("z_mu_n2", "z_logvar_n2", "mu_Beta2"),
]
MU_NAMES = ["mu_Alpha", "mu_Beta", "mu_Alpha1", "mu_Beta1", "mu_Alpha2", "mu_Beta2"]
Q_NAMES = ["Alpha_mu", "Beta_mu", "Alpha_mu1", "Beta_mu1", "Alpha_mu2", "Beta_mu2"]


def _build_program(KH: int):
    """Build + compile the single-core SPMD Bass program.  KH = padded max
    edge count per row; U = 16*KH is the per-row-tile gather width."""
    U = 16 * KH
    nc = bacc.Bacc("TRN2", target_bir_lowering=False, debug=False, num_devices=M)

    x_d = nc.dram_tensor("x", [R, N], f32, kind="ExternalInput").ap()
    eidx_d = nc.dram_tensor("eidx", [128, RT * KH], i16, kind="ExternalInput").ap()
    iota_d = nc.dram_tensor("iota2", [128, U], fp16, kind="ExternalInput").ap()
    cnt_d = nc.dram_tensor("cnt", [128, RT], f32, kind="ExternalInput").ap()
    zm_d = nc.dram_tensor("zm", [128, ZW], f32, kind="ExternalInput").ap()
    zl_d = nc.dram_tensor("zl", [128, ZW], f32, kind="ExternalInput").ap()
    pm_d = nc.dram_tensor("pm", [128, ZW], f32, kind="ExternalInput").ap()
    vec_d = nc.dram_tensor("vec", [1, 768], f32, kind="ExternalInput").ap()
    # per-partition partial sums, reduced on host
    oa_d = nc.dram_tensor("out_act", [128, RT + ZC], f32, kind="ExternalOutput").ap()
    od_d = nc.dram_tensor("out_dve", [128, RT + 2 * ZC], f32,
                          kind="ExternalOutput").ap()
    ov_d = nc.dram_tensor("out_vec", [1, 2], f32, kind="ExternalOutput").ap()

    AF = mybir.ActivationFunctionType
    OP = mybir.AluOpType

    # Exp and Ln both live in the natural_log_exp_and_others table set; drop
    # them from the single-function sets so the act-table pass picks the
    # shared set instead of reloading tables between Exp and Ln (set order —
    # and therefore every act_func_set_id — is unchanged).
    tabs = hw_specs.get_activation_tables(nc.m.arch)
    if "natural_log_exp_and_others" in tabs:
        tabs["exp_and_others"].discard(AF.Exp)
        tabs["natural_log"].discard(AF.Ln)

    with tile.TileContext(nc) as tc:
        with (
            tc.tile_pool(name="xp", bufs=2) as xp,
            tc.tile_pool(name="ep", bufs=3) as ep,
            tc.tile_pool(name="up", bufs=2) as up,
            tc.tile_pool(name="gp", bufs=2) as gp,
            tc.tile_pool(name="sp", bufs=1) as sp,
            tc.tile_pool(name="tp", bufs=1) as tp,
            tc.tile_pool(name="zp", bufs=1) as zp,
            tc.tile_pool(name="cst", bufs=1) as cst,
        ):
            acc_a = cst.tile([128, RT + ZC], f32)
            acc_d = cst.tile([128, RT + 2 * ZC], f32)
            cnt = cst.tile([128, RT], f32)
            nc.sync.dma_start(cnt[:], cnt_d[:])
            iota2 = cst.tile([128, U], fp16)
            nc.sync.dma_start(iota2[:], iota_d[:])
            eidx = cst.tile([128, RT * KH], i16)
            nc.sync.dma_start(eidx[:], eidx_d[:])

            # ---- BCE over the row-sharded edge_logits block ----
            for t in range(RT):
                xt = xp.tile([128, N], f32)
                nc.sync.dma_start(xt[:], x_d[t * 128:(t + 1) * 128, :])
                # dense: e = exp(x) in two halves
                elo = ep.tile([128, H], bf16, tag="e")
                nc.scalar.activation(elo[:], xt[:, 0:H], AF.Exp)
                ehi = ep.tile([128, H], bf16, tag="e")
                nc.scalar.activation(ehi[:], xt[:, H:N], AF.Exp)
                # edge: gather + range-mask + masked dot
                g = gp.tile([128, U], f32)
                nc.gpsimd.ap_gather(g[:], xt[:], eidx[:, t * KH:(t + 1) * KH],
                                    channels=128, num_elems=N, d=1, num_idxs=U)
                sel = sp.tile([128, U], fp16)
                nc.gpsimd.tensor_scalar(sel[:], iota2[:], cnt[:, t:t + 1], None,
                                        OP.is_lt)
                td = tp.tile([128, U], bf16, tag="t")
                nc.vector.scalar_tensor_tensor(
                    td[:], g[:], 0.0, sel[:], OP.bypass, OP.mult,
                    accum_out=acc_d[:, t:t + 1])
                # dense pairing: u = e_lo + e_hi + e_lo*e_hi = (1+e_lo)(1+e_hi)-1
                pt = tp.tile([128, H], bf16, tag="t")
                nc.vector.scalar_tensor_tensor(
                    pt[:], elo[:], 1.0, ehi[:], OP.add, OP.mult)
                ut = up.tile([128, H], bf16)
                nc.vector.tensor_add(ut[:], pt[:], elo[:])
                # dense reduce: softplus sum = sum ln(u + 1)
                lo = zp.tile([128, H], bf16, tag="atrash")
                nc.scalar.activation(lo[:], ut[:], AF.Ln, bias=1.0,
                                     accum_out=acc_a[:, t:t + 1])

            # ---- per-node KL partial sums over the packed z tensors ----
            for zc in range(ZC):
                s = slice(zc * ZWc, (zc + 1) * ZWc)
                zm = zp.tile([128, ZWc], f32, tag="zm")
                nc.sync.dma_start(zm[:], zm_d[:, s])
                zl = zp.tile([128, ZWc], f32, tag="zl")
                nc.sync.dma_start(zl[:], zl_d[:, s])
                pm = zp.tile([128, ZWc], f32, tag="pm")
                nc.sync.dma_start(pm[:], pm_d[:, s])
                df = zp.tile([128, ZWc], f32, tag="df")
                nc.vector.tensor_sub(df[:], zm[:], pm[:])
                ztr = zp.tile([128, ZWc], bf16, tag="ztr")
                nc.vector.scalar_tensor_tensor(
                    ztr[:], df[:], 0.0, df[:], OP.bypass, OP.mult,
                    accum_out=acc_d[:, RT + zc:RT + zc + 1])
                nc.vector.tensor_scalar(
                    ztr[:], zl[:], 1.0, None, OP.mult, OP.add,
                    accum_out=acc_d[:, RT + ZC + zc:RT + ZC + zc + 1])
                zex = zp.tile([128, ZWc], bf16, tag="atrash")
                nc.scalar.activation(zex[:], zl[:], AF.Exp,
                                     accum_out=acc_a[:, RT + zc:RT + zc + 1])

            # ---- prior/extra-KL terms over the tiny mu vectors (DVE) ----
            vb = cst.tile([1, 2048], f32)   # [vec 768 | vtr 384 | vdf 384 | accv 2]
            nc.sync.dma_start(vb[:, 0:768], vec_d[:])
            nc.vector.scalar_tensor_tensor(
                vb[:, 768:1152], vb[:, 0:384], 0.0, vb[:, 0:384],
                OP.bypass, OP.mult, accum_out=vb[:, 1536:1537])
            nc.vector.tensor_sub(vb[:, 1152:1536], vb[:, 0:384], vb[:, 384:768])
            nc.vector.scalar_tensor_tensor(
                vb[:, 768:1152], vb[:, 1152:1536], 0.0, vb[:, 1152:1536],
                OP.bypass, OP.mult, accum_out=vb[:, 1537:1538])

            nc.sync.dma_start(oa_d[:], acc_a[:])
            nc.sync.dma_start(od_d[:], acc_d[:])
            nc.sync.dma_start(ov_d[:], vb[:, 1536:1538])

    nc.compile()
    return nc


def _prepare(inputs: dict):
    """Host-side sharding: slice the big tensors per core, symmetrize + dedup
    the edge list into per-row padded column lists (int16) for the on-device
    gather, plus the fixed-segment iota/count tensors for the validity mask."""
    x = np.ascontiguousarray(np.asarray(inputs["edge_logits"], dtype=np.float32))
    assert x.shape == (N, N)
    ei = np.asarray(inputs["edge_index"]).astype(np.int64)

    # symmetric adjacency with set() semantics -> deduplicated edge keys
    k1 = ei[0] * N + ei[1]
    k2 = ei[1] * N + ei[0]
    u = np.unique(np.concatenate([k1, k2]))  # sorted
    rows = (u >> 13).astype(np.int64)
    cols = (u & (N - 1)).astype(np.int64)

    cnt_row = np.bincount(rows, minlength=N)
    KH = int(cnt_row.max())
    KH = max(2, (KH + 1) // 2 * 2)
    first = np.zeros(N, np.int64)
    np.cumsum(cnt_row[:-1], out=first[1:])
    pos = np.arange(u.size) - first[rows]
    idx_pad = np.zeros((N, KH), np.int16)
    idx_pad[rows, pos] = cols.astype(np.int16)

    # wrapped 16-partition-group gather lists:
    # partition 16q+p, free (tile, k) holds L_group[k*16+p]
    A = idx_pad.reshape(M, RT, 8, 16 * KH)          # [c][t][q][pl-major list]
    B = A.reshape(M, RT, 8, KH, 16)                 # B[..., k, p] = L[k*16+p]
    eidx_all = np.ascontiguousarray(B.transpose(0, 2, 4, 1, 3)).reshape(
        M, 128, RT * KH)

    # iota2[p, j] = j - (p%16)*KH inside partition p's own segment, else big
    U16 = 16 * KH
    j = np.arange(U16)
    seg = (np.arange(128) % 16) * KH
    io2 = np.full((128, U16), FP16_PAD, np.float32)
    rel = j[None, :] - seg[:, None]
    own = (rel >= 0) & (rel < KH)
    io2[own] = rel[own]
    io2 = io2.astype(np.float16)

    cnt_all = np.ascontiguousarray(
        cnt_row.reshape(M, RT, 128).transpose(0, 2, 1)).astype(np.float32)

    def pack_z(name):
        # (8192, 64) -> [core][128][512]; partition p holds rows 8p..8p+7
        return np.asarray(inputs[name], dtype=np.float32).reshape(M, R // 8, 8 * D)

    zm_all = np.ascontiguousarray(
        np.concatenate([pack_z(zm) for zm, _, _ in Z_PAIRS], axis=2))
    zl_all = np.ascontiguousarray(
        np.concatenate([pack_z(zl) for _, zl, _ in Z_PAIRS], axis=2))
    pm_all = np.concatenate(
        [np.tile(np.asarray(inputs[p], np.float32), 8) for _, _, p in Z_PAIRS])
    pm_all = np.ascontiguousarray(
        np.broadcast_to(pm_all[None, :], (128, ZW)).astype(np.float32))

    mu_cat = np.concatenate([np.asarray(inputs[n], np.float32) for n in MU_NAMES])
    q_cat = np.concatenate([np.asarray(inputs[n], np.float32) for n in Q_NAMES])
    vec = np.concatenate([mu_cat, q_cat]).reshape(1, 768).astype(np.float32)

    in_maps = []
    for c in range(M):
        in_maps.append({
            "x": np.ascontiguousarray(x[c * R:(c + 1) * R]),
            "eidx": eidx_all[c],
            "iota2": io2,
            "cnt": cnt_all[c],
            "zm": zm_all[c],
            "zl": zl_all[c],
            "pm": pm_all,
            "vec": vec,
        })
    return in_maps, KH


def _combine(results: list) -> np.float32:
    """Host-side all-reduce of the per-core per-partition partial sums."""
    SP = 0.0   # sum softplus(x)
    ED = 0.0   # sum x * adj
    ZSQ = 0.0  # sum (z_mu - mu)^2
    ZEX = 0.0  # sum exp(z_logvar)
    ZL = 0.0   # sum z_logvar
    for r in results:
        oa = np.asarray(r["out_act"], np.float64)
        od = np.asarray(r["out_dve"], np.float64)
        SP += oa[:, 0:RT].sum()
        ZEX += oa[:, RT:RT + ZC].sum()
        ED += od[:, 0:RT].sum()
        ZSQ += od[:, RT:RT + ZC].sum()
        ZL += od[:, RT + ZC:RT + 2 * ZC].sum()
    ov = np.asarray(results[0]["out_vec"], np.float64)
    V1, V2 = ov[0, 0], ov[0, 1]

    log_p = 6.0 * (-0.5 * LOG2PI) - 0.5 * V1 / D
    extra_kl = 2.0 * V2 / D
    logpx_z = (SP - ED) / (float(N) * float(N))
    kl = 6.0 * (-0.5 + 0.5 * LV) + (-0.5 * ZL + 2.0 * ZSQ + 2.0 * ZEX) / (N * D)
    return np.float32(log_p + extra_kl + logpx_z + kl)


def kernel(**inputs) -> np.ndarray:
    in_maps, KH = _prepare(inputs)
    if KH not in _PROG_CACHE:
        _PROG_CACHE[KH] = _build_program(KH)
    nc = _PROG_CACHE[KH]
    res = run_bass_kernel_spmd(nc, in_maps, core_ids=list(range(M)))
    return _combine(res.results)


if __name__ == "__main__":
    # smoke test with random data
    rng = np.random.default_rng(0)
    demo = {nm: rng.standard_normal((N, D)).astype(np.float32)
            for nm in [z for p in Z_PAIRS for z in p[:2]]}
    for nm in MU_NAMES + Q_NAMES:
        demo[nm] = rng.standard_normal(D).astype(np.float32)
    demo["edge_logits"] = rng.standard_normal((N, N)).astype(np.float32)
    demo["edge_index"] = rng.integers(0, N, (2, 262144)).astype(np.int64)
    demo["num_nodes"] = N
    print(kernel(**demo))


# revision 6
# speedup vs baseline: 1.4707x; 1.4707x over previous
"""Trainium2 Bass kernel for the nn_HVAE loss function, SPMD over 8 NeuronCores.

Sharding: edge_logits (8192x8192) and the z_* (8192x64) tensors are row-sharded
over nodes across the 8 cores.  Per core:

  BCE dense term   sum(softplus(x)) = sum(ln(1 + e^x)): ScalarE computes
                   e = exp(x); VectorE pairs columns (1+e_lo)(1+e_hi) - 1 so
                   ScalarE's Ln(u + 1) pass runs on half the elements; the
                   per-partition sums come from the activation accumulator.
                   Exp and Ln share one activation table set
                   (natural_log_exp_and_others) to avoid table reloads.
  BCE edge term    sum(x * adj): the host dedups + symmetrizes edge_index into
                   per-row column lists; GPSIMD ap_gather pulls the edge
                   entries of x into a compact [128, 16*KH] tile (16-row
                   groups, fixed KH-wide per-row segments), a tensor_scalar
                   range-compare builds the validity mask, and VectorE does the
                   masked dot-product + reduction.
  KL terms         over the row-sharded z tensors on ScalarE/VectorE.

Each core writes per-partition partial sums; the host does the final (tiny)
all-reduce of the scalar partials in float64.
"""

import sys

import numpy as np

sys.path.insert(0, "/opt/trn_rl_repo")

import concourse.bass as bass  # noqa: E402
import concourse.tile as tile  # noqa: E402
from concourse import bacc, hw_specs, mybir  # noqa: E402
from concourse.bass_utils import run_bass_kernel_spmd  # noqa: E402

N = 8192          # num nodes
D = 64            # latent dim
M = 8             # cores
R = N // M        # 1024 rows per core
RT = R // 128     # 8 row tiles of 128 partitions per core
H = N // 2        # 4096: pairing half-width
ZW = 6 * (R // 128) * D   # 3072: six (1024,64) pairs packed as [128, 512] each
ZC = 3            # z chunks
ZWc = ZW // ZC    # 1024
FP16_PAD = 32768.0  # iota2 fill for foreign segments (never < count)
LOG2PI = float(np.log(2.0 * np.pi))
LV = float(np.log(0.25))

f32 = mybir.dt.float32
bf16 = mybir.dt.bfloat16
fp16 = mybir.dt.float16
i16 = mybir.dt.int16

_PROG_CACHE: dict = {}

Z_PAIRS = [
    ("z_mu_e", "z_logvar_e", "mu_Alpha"),
    ("z_mu_n", "z_logvar_n", "mu_Beta"),
    ("z_mu_e1", "z_logvar_e1", "mu_Alpha1"),
    ("z_mu_n1", "z_logvar_n1", "mu_Beta1"),
    ("z_mu_e2", "z_logvar_e2", "mu_Alpha2"),
    ("z_mu_n2", "z_logvar_n2", "mu_Beta2"),
]
MU_NAMES = ["mu_Alpha", "mu_Beta", "mu_Alpha1", "mu_Beta1", "mu_Alpha2", "mu_Beta2"]
Q_NAMES = ["Alpha_mu", "Beta_mu", "Alpha_mu1", "Beta_mu1", "Alpha_mu2", "Beta_mu2"]


def _build_program(KH: int):
    """Build + compile the single-core SPMD Bass program.  KH = padded max
    edge count per row; U = 16*KH is the per-row-tile gather width."""
    U = 16 * KH
    nc = bacc.Bacc("TRN2", target_bir_lowering=False, debug=False, num_devices=M)

    x_d = nc.dram_tensor("x", [R, N], f32, kind="ExternalInput").ap()
    eidx_d = nc.dram_tensor("eidx", [128, RT * KH], i16, kind="ExternalInput").ap()
    iota_d = nc.dram_tensor("iota2", [128, U], fp16, kind="ExternalInput").ap()
    cnt_d = nc.dram_tensor("cnt", [128, RT], f32, kind="ExternalInput").ap()
    zm_d = nc.dram_tensor("zm", [128, ZW], f32, kind="ExternalInput").ap()
    zl_d = nc.dram_tensor("zl", [128, ZW], f32, kind="ExternalInput").ap()
    pm_d = nc.dram_tensor("pm", [128, ZW], f32, kind="ExternalInput").ap()
    vec_d = nc.dram_tensor("vec", [1, 768], f32, kind="ExternalInput").ap()
    # per-partition partial sums, reduced on host
    oa_d = nc.dram_tensor("out_act", [128, RT + ZC], f32, kind="ExternalOutput").ap()
    od_d = nc.dram_tensor("out_dve", [128, RT + 2 * ZC], f32,
                          kind="ExternalOutput").ap()
    ov_d = nc.dram_tensor("out_vec", [1, 2], f32, kind="ExternalOutput").ap()

    AF = mybir.ActivationFunctionType
    OP = mybir.AluOpType

    # Exp and Ln both live in the natural_log_exp_and_others table set; drop
    # them from the single-function sets so the act-table pass picks the
    # shared set instead of reloading tables between Exp and Ln (set order —
    # and therefore every act_func_set_id — is unchanged).
    tabs = hw_specs.get_activation_tables(nc.m.arch)
    if "natural_log_exp_and_others" in tabs:
        tabs["exp_and_others"].discard(AF.Exp)
        tabs["natural_log"].discard(AF.Ln)

    with tile.TileContext(nc) as tc:
        with (
            tc.tile_pool(name="xp", bufs=2) as xp,
            tc.tile_pool(name="ep", bufs=3) as ep,
            tc.tile_pool(name="up", bufs=2) as up,
            tc.tile_pool(name="gp", bufs=2) as gp,
            tc.tile_pool(name="sp", bufs=1) as sp,
            tc.tile_pool(name="tp", bufs=1) as tp,
            tc.tile_pool(name="zp", bufs=1) as zp,
            tc.tile_pool(name="cst", bufs=1) as cst,
        ):
            acc_a = cst.tile([128, RT + ZC], f32)
            acc_d = cst.tile([128, RT + 2 * ZC], f32)
            cnt = cst.tile([128, RT], f32)
            nc.sync.dma_start(cnt[:], cnt_d[:])
            iota2 = cst.tile([128, U], fp16)
            nc.sync.dma_start(iota2[:], iota_d[:])
            eidx = cst.tile([128, RT * KH], i16)
            nc.sync.dma_start(eidx[:], eidx_d[:])

            # ---- BCE over the row-sharded edge_logits block ----
            for t in range(RT):
                xt = xp.tile([128, N], f32)
                nc.sync.dma_start(xt[:], x_d[t * 128:(t + 1) * 128, :])
                # dense: e = exp(x) in two halves
                elo = ep.tile([128, H], bf16, tag="e")
                nc.scalar.activation(elo[:], xt[:, 0:H], AF.Exp)
                ehi = ep.tile([128, H], bf16, tag="e")
                nc.scalar.activation(ehi[:], xt[:, H:N], AF.Exp)
                # edge: gather + range-mask + masked dot
                g = gp.tile([128, U], f32)
                nc.gpsimd.ap_gather(g[:], xt[:], eidx[:, t * KH:(t + 1) * KH],
                                    channels=128, num_elems=N, d=1, num_idxs=U)
                sel = sp.tile([128, U], fp16)
                nc.vector.tensor_scalar(sel[:], iota2[:], cnt[:, t:t + 1], None,
                                        OP.is_lt)
                td = tp.tile([128, U], bf16, tag="t")
                nc.vector.scalar_tensor_tensor(
                    td[:], g[:], 0.0, sel[:], OP.bypass, OP.mult,
                    accum_out=acc_d[:, t:t + 1])
                # dense pairing: u = e_lo + e_hi + e_lo*e_hi = (1+e_lo)(1+e_hi)-1
                pt = tp.tile([128, H], bf16, tag="t")
                nc.vector.scalar_tensor_tensor(
                    pt[:], elo[:], 1.0, ehi[:], OP.add, OP.mult)
                ut = up.tile([128, H], bf16)
                nc.vector.tensor_add(ut[:], pt[:], elo[:])
                # dense reduce: softplus sum = sum ln(u + 1)
                lo = zp.tile([128, H], bf16, tag="atrash")
                nc.scalar.activation(lo[:], ut[:], AF.Ln, bias=1.0,
                                     accum_out=acc_a[:, t:t + 1])

            # ---- per-node KL partial sums over the packed z tensors ----
            for zc in range(ZC):
                s = slice(zc * ZWc, (zc + 1) * ZWc)
                zm = zp.tile([128, ZWc], f32, tag="zm")
                nc.sync.dma_start(zm[:], zm_d[:, s])
                zl = zp.tile([128, ZWc], f32, tag="zl")
                nc.sync.dma_start(zl[:], zl_d[:, s])
                pm = zp.tile([128, ZWc], f32, tag="pm")
                nc.sync.dma_start(pm[:], pm_d[:, s])
                df = zp.tile([128, ZWc], f32, tag="df")
                nc.vector.tensor_sub(df[:], zm[:], pm[:])
                ztr = zp.tile([128, ZWc], bf16, tag="ztr")
                nc.vector.scalar_tensor_tensor(
                    ztr[:], df[:], 0.0, df[:], OP.bypass, OP.mult,
                    accum_out=acc_d[:, RT + zc:RT + zc + 1])
                nc.vector.tensor_scalar(
                    ztr[:], zl[:], 1.0, None, OP.mult, OP.add,
                    accum_out=acc_d[:, RT + ZC + zc:RT + ZC + zc + 1])
                zex = zp.tile([128, ZWc], bf16, tag="atrash")
                nc.scalar.activation(zex[:], zl[:], AF.Exp,
                                     accum_out=acc_a[:, RT + zc:RT + zc + 1])

            # ---- prior/extra-KL terms over the tiny mu vectors (DVE) ----
            vb = cst.tile([1, 2048], f32)   # [vec 768 | vtr 384 | vdf 384 | accv 2]
            nc.sync.dma_start(vb[:, 0:768], vec_d[:])
            nc.vector.scalar_tensor_tensor(
                vb[:, 768:1152], vb[:, 0:384], 0.0, vb[:, 0:384],
                OP.bypass, OP.mult, accum_out=vb[:, 1536:1537])
            nc.vector.tensor_sub(vb[:, 1152:1536], vb[:, 0:384], vb[:, 384:768])
            nc.vector.scalar_tensor_tensor(
                vb[:, 768:1152], vb[:, 1152:1536], 0.0, vb[:, 1152:1536],
                OP.bypass, OP.mult, accum_out=vb[:, 1537:1538])

            nc.sync.dma_start(oa_d[:], acc_a[:])
            nc.sync.dma_start(od_d[:], acc_d[:])
            nc.sync.dma_start(ov_d[:], vb[:, 1536:1538])

    nc.compile()
    return nc


def _prepare(inputs: dict):
    """Host-side sharding: slice the big tensors per core, symmetrize + dedup
    the edge list into per-row padded column lists (int16) for the on-device
    gather, plus the fixed-segment iota/count tensors for the validity mask."""
    x = np.ascontiguousarray(np.asarray(inputs["edge_logits"], dtype=np.float32))
    assert x.shape == (N, N)
    ei = np.asarray(inputs["edge_index"]).astype(np.int64)

    # symmetric adjacency with set() semantics -> deduplicated edge keys
    k1 = ei[0] * N + ei[1]
    k2 = ei[1] * N + ei[0]
    u = np.unique(np.concatenate([k1, k2]))  # sorted
    rows = (u >> 13).astype(np.int64)
    cols = (u & (N - 1)).astype(np.int64)

    cnt_row = np.bincount(rows, minlength=N)
    KH = int(cnt_row.max())
    KH = max(2, (KH + 1) // 2 * 2)
    first = np.zeros(N, np.int64)
    np.cumsum(cnt_row[:-1], out=first[1:])
    pos = np.arange(u.size) - first[rows]
    idx_pad = np.zeros((N, KH), np.int16)
    idx_pad[rows, pos] = cols.astype(np.int16)

    # wrapped 16-partition-group gather lists:
    # partition 16q+p, free (tile, k) holds L_group[k*16+p]
    A = idx_pad.reshape(M, RT, 8, 16 * KH)          # [c][t][q][pl-major list]
    B = A.reshape(M, RT, 8, KH, 16)                 # B[..., k, p] = L[k*16+p]
    eidx_all = np.ascontiguousarray(B.transpose(0, 2, 4, 1, 3)).reshape(
        M, 128, RT * KH)

    # iota2[p, j] = j - (p%16)*KH inside partition p's own segment, else big
    U16 = 16 * KH
    j = np.arange(U16)
    seg = (np.arange(128) % 16) * KH
    io2 = np.full((128, U16), FP16_PAD, np.float32)
    rel = j[None, :] - seg[:, None]
    own = (rel >= 0) & (rel < KH)
    io2[own] = rel[own]
    io2 = io2.astype(np.float16)

    cnt_all = np.ascontiguousarray(
        cnt_row.reshape(M, RT, 128).transpose(0, 2, 1)).astype(np.float32)

    def pack_z(name):
        # (8192, 64) -> [core][128][512]; partition p holds rows 8p..8p+7
        return np.asarray(inputs[name], dtype=np.float32).reshape(M, R // 8, 8 * D)

    zm_all = np.ascontiguousarray(
        np.concatenate([pack_z(zm) for zm, _, _ in Z_PAIRS], axis=2))
    zl_all = np.ascontiguousarray(
        np.concatenate([pack_z(zl) for _, zl, _ in Z_PAIRS], axis=2))
    pm_all = np.concatenate(
        [np.tile(np.asarray(inputs[p], np.float32), 8) for _, _, p in Z_PAIRS])
    pm_all = np.ascontiguousarray(
        np.broadcast_to(pm_all[None, :], (128, ZW)).astype(np.float32))

    mu_cat = np.concatenate([np.asarray(inputs[n], np.float32) for n in MU_NAMES])
    q_cat = np.concatenate([np.asarray(inputs[n], np.float32) for n in Q_NAMES])
    vec = np.concatenate([mu_cat, q_cat]).reshape(1, 768).astype(np.float32)

    in_maps = []
    for c in range(M):
        in_maps.append({
            "x": np.ascontiguousarray(x[c * R:(c + 1) * R]),
            "eidx": eidx_all[c],
            "iota2": io2,
            "cnt": cnt_all[c],
            "zm": zm_all[c],
            "zl": zl_all[c],
            "pm": pm_all,
            "vec": vec,
        })
    return in_maps, KH


def _combine(results: list) -> np.float32:
    """Host-side all-reduce of the per-core per-partition partial sums."""
    SP = 0.0   # sum softplus(x)
    ED = 0.0   # sum x * adj
    ZSQ = 0.0  # sum (z_mu - mu)^2
    ZEX = 0.0  # sum exp(z_logvar)
    ZL = 0.0   # sum z_logvar
    for r in results:
        oa = np.asarray(r["out_act"], np.float64)
        od = np.asarray(r["out_dve"], np.float64)
        SP += oa[:, 0:RT].sum()
        ZEX += oa[:, RT:RT + ZC].sum()
        ED += od[:, 0:RT].sum()
        ZSQ += od[:, RT:RT + ZC].sum()
        ZL += od[:, RT + ZC:RT + 2 * ZC].sum()
    ov = np.asarray(results[0]["out_vec"], np.float64)
    V1, V2 = ov[0, 0], ov[0, 1]

    log_p = 6.0 * (-0.5 * LOG2PI) - 0.5 * V1 / D
    extra_kl = 2.0 * V2 / D
    logpx_z = (SP - ED) / (float(N) * float(N))
    kl = 6.0 * (-0.5 + 0.5 * LV) + (-0.5 * ZL + 2.0 * ZSQ + 2.0 * ZEX) / (N * D)
    return np.float32(log_p + extra_kl + logpx_z + kl)


def kernel(**inputs) -> np.ndarray:
    in_maps, KH = _prepare(inputs)
    if KH not in _PROG_CACHE:
        _PROG_CACHE[KH] = _build_program(KH)
    nc = _PROG_CACHE[KH]
    res = run_bass_kernel_spmd(nc, in_maps, core_ids=list(range(M)))
    return _combine(res.results)


if __name__ == "__main__":
    # smoke test with random data
    rng = np.random.default_rng(0)
    demo = {nm: rng.standard_normal((N, D)).astype(np.float32)
            for nm in [z for p in Z_PAIRS for z in p[:2]]}
    for nm in MU_NAMES + Q_NAMES:
        demo[nm] = rng.standard_normal(D).astype(np.float32)
    demo["edge_logits"] = rng.standard_normal((N, N)).astype(np.float32)
    demo["edge_index"] = rng.integers(0, N, (2, 262144)).astype(np.int64)
    demo["num_nodes"] = N
    print(kernel(**demo))
